# revision 24
# baseline (speedup 1.0000x reference)
"""SchNet CFConv kernel for 8 TRN2 NeuronCores (Bass/Tile).

Math (per batch b, atom n, neighbor slot k):
    W   = ssp(f_ij @ Wf1 + bf1) @ Wf2 + bf2          ssp(v) = softplus(v) - ln2
    y   = x @ Win
    out = ssp( (sum_k mask * W * y[nbr]) @ Wout + bout )

Device strategy (data-parallel over batch, 2 molecules per core):
  * Host pre-transposes f_ij to fT [G, pairs] bf16 so the filter matmuls run
    with G on partitions (no on-device transposes).
  * ssp(v) = ln(0.5*exp(v)+0.5) exactly (no Softplus table in this
    toolchain; Exp/Ln/Abs/Copy share the natural_log_exp table set). The
    final layer uses the stable relu(z) + ln(0.5*exp(-|z|)+0.5) form.
  * Neighbor gather = one-hot matmul on TensorE (gather_mode="onehot",
    default): host uploads the neighbor index row replicated across 128
    partitions as bf16 (masked pairs -> 300, matches nothing). DVE builds
    one-hot tiles with is_equal against per-partition iota constants
    (rows 0-127 / 128-255), and two accumulated K=128 matmuls against the
    per-batch y half-tables select y[nbr] exactly into PSUM. This avoids
    gpsimd dma_gather entirely, whose single-Q7-pair descriptor generation
    (~14 ns/idx) dominated the old kernel (~1.9 ms of the 2.0 ms total).
  * gather_mode="dma" keeps the old dma_gather path for comparison.
"""

import math
import os
from contextlib import ExitStack

import ml_dtypes
import numpy as np

import concourse.bass as bass
import concourse.mybir as mybir
import concourse.tile as tile
from concourse import bacc, library_config
from concourse.bass_utils import run_bass_kernel_spmd

BF16 = ml_dtypes.bfloat16
LOG2 = float(np.log(2.0))

B, N, NBH, G, F = 16, 256, 255, 50, 128
NCORES = 8
BPC = B // NCORES          # batches (molecules) per core
PAIRS_B = N * NBH          # 65280 pairs per batch
ATOMS_PER_GROUP = 2
GROUP = ATOMS_PER_GROUP * NBH   # 510 pairs per group
NG_B = PAIRS_B // GROUP         # 128 groups per batch
IDXW = 512                      # gather idxs per group (510 real + 2 pad)
IDXC = IDXW // 16               # idx columns per group in the [16, *] layout

FP32 = mybir.dt.float32
BF16D = mybir.dt.bfloat16
I16 = mybir.dt.int16


def build_nc(n_batch=BPC, n_atoms=N, repeat=1, gather_mode="onehot",
             single_packet=False, ssp_mode="expln", n_queues=1, gblk=16):
    """Build the per-core Bass program. Parametric so CoreSim can run tiny.

    ssp_mode: "expln" (production, 2 ACT passes), "zero" (no ACT in main
    loop, timing experiment only).
    """
    assert n_atoms % ATOMS_PER_GROUP == 0
    pairs_b = n_atoms * NBH
    ng_b = pairs_b // GROUP           # groups per batch
    n_rows = n_batch * n_atoms        # y-table rows (+1 zero row for dma mode)
    zrow_id = n_rows
    cpb = (n_atoms + 127) // 128      # 128-row y chunks per batch (onehot)

    nc = bacc.Bacc(None, target_bir_lowering=False, num_swdge_queues=n_queues)

    fT = nc.declare_dram_parameter("fT", [G, n_batch * pairs_b], BF16D, False)
    xT = nc.declare_dram_parameter("xT", [F, n_rows], BF16D, False)
    wf1 = nc.declare_dram_parameter("wf1", [G, F], BF16D, False)
    wf2 = nc.declare_dram_parameter("wf2", [F, F], BF16D, False)
    win = nc.declare_dram_parameter("win", [F, F], BF16D, False)
    wout = nc.declare_dram_parameter("wout", [F, F], BF16D, False)
    bf1 = nc.declare_dram_parameter("bf1", [F, 1], FP32, False)
    bf2p = nc.declare_dram_parameter("bf2p", [F, 1], FP32, False)
    bout = nc.declare_dram_parameter("bout", [1, F], BF16D, False)
    if gather_mode == "onehot":
        idxbc = nc.declare_dram_parameter(
            "idxbc", [128, n_batch * pairs_b], BF16D, False
        )
        iota = nc.declare_dram_parameter("iota", [128, cpb], FP32, False)
    else:
        idx = nc.declare_dram_parameter(
            "idx", [128, n_batch * ng_b * IDXC], I16, False
        )
    out = nc.declare_dram_parameter("out", [n_batch, n_atoms, F], FP32, isOutput=True)

    with tile.TileContext(nc) as tc, ExitStack() as ctx:
        consts = ctx.enter_context(tc.tile_pool(name="consts", bufs=1))
        misc = ctx.enter_context(tc.tile_pool(name="misc", bufs=4))
        ftp = ctx.enter_context(tc.tile_pool(name="ftp", bufs=2))
        esp = ctx.enter_context(tc.tile_pool(name="esp", bufs=2))
        actp = ctx.enter_context(tc.tile_pool(name="actp", bufs=4))
        sttp = ctx.enter_context(tc.tile_pool(name="sttp", bufs=2))
        ps1p = ctx.enter_context(tc.tile_pool(name="ps1p", bufs=1, space="PSUM"))
        ps2p = ctx.enter_context(tc.tile_pool(name="ps2p", bufs=1, space="PSUM"))
        ycolp = ctx.enter_context(tc.tile_pool(name="ycolp", bufs=2))
        yfinp = ctx.enter_context(tc.tile_pool(name="yfinp", bufs=2))
        if gather_mode == "onehot":
            ibcp = ctx.enter_context(tc.tile_pool(name="ibcp", bufs=2))
            ohp = ctx.enter_context(tc.tile_pool(name="ohp", bufs=2 * cpb))
            ynpp = ctx.enter_context(tc.tile_pool(name="ynpp", bufs=2, space="PSUM"))
            yhp = ctx.enter_context(tc.tile_pool(name="yhp", bufs=n_batch * cpb))
        else:
            dram = ctx.enter_context(tc.tile_pool(name="dram", bufs=1, space="DRAM"))
            psmisc = ctx.enter_context(tc.tile_pool(name="psmisc", bufs=1, space="PSUM"))
            ynbp = ctx.enter_context(tc.tile_pool(name="ynbp", bufs=3))
            nc.gpsimd.load_library(library_config.mlp)

        # ---- constants into SBUF ----
        wf1_sb = consts.tile([G, F], BF16D)
        nc.sync.dma_start(out=wf1_sb[:], in_=wf1[:])
        wf2_sb = consts.tile([F, F], BF16D)
        nc.sync.dma_start(out=wf2_sb[:], in_=wf2[:])
        win_sb = consts.tile([F, F], BF16D)
        nc.sync.dma_start(out=win_sb[:], in_=win[:])
        wout_sb = consts.tile([F, F], BF16D)
        nc.sync.dma_start(out=wout_sb[:], in_=wout[:])
        bf1_sb = consts.tile([F, 1], FP32)
        nc.sync.dma_start(out=bf1_sb[:], in_=bf1[:])
        bf2p_sb = consts.tile([F, 1], FP32)
        nc.sync.dma_start(out=bf2p_sb[:], in_=bf2p[:])
        bout_sb = consts.tile([1, F], BF16D)
        nc.sync.dma_start(out=bout_sb[:], in_=bout[:])
        xT_sb = consts.tile([F, n_rows], BF16D)
        nc.sync.dma_start(out=xT_sb[:], in_=xT[:])
        ones_sb = consts.tile([1, F], BF16D)
        nc.vector.memset(ones_sb[:], 1.0)
        half_sb = consts.tile([F, 1], FP32)
        nc.vector.memset(half_sb[:], 0.5)
        if gather_mode == "onehot":
            iota_sb = consts.tile([128, cpb], FP32)
            nc.sync.dma_start(out=iota_sb[:], in_=iota[:])
        else:
            zrow_sb = consts.tile([1, F], BF16D)
            nc.vector.memset(zrow_sb[:], 0.0)
            idx_sb = consts.tile([128, n_batch * ng_b * IDXC], I16)
            nc.sync.dma_start(out=idx_sb[:], in_=idx[:])

        sblk = min(2, ng_b)   # groups per ssp supertile
        gblk = min(gblk, ng_b)  # groups per stream block
        assert ng_b % gblk == 0 and gblk % sblk == 0

        psz_pool = ps2p if gather_mode == "onehot" else psmisc
        psz_tag = "ps2" if gather_mode == "onehot" else "pmisc"

        def f2out(b, ycols):
            # out[b] = ssp(ycols.T @ Wout + bout)
            yfin = yfinp.tile([F, n_atoms], BF16D)
            nc.vector.tensor_copy(out=yfin[:], in_=ycols[:])
            for h0 in range(0, n_atoms, 128):
                m = min(128, n_atoms - h0)
                psz = psz_pool.tile([128, F], FP32, tag=psz_tag, name="psz")
                nc.tensor.matmul(
                    out=psz[:m, :],
                    lhsT=yfin[:, h0 : h0 + m],
                    rhs=wout_sb[:],
                    start=True,
                    stop=False,
                )
                nc.tensor.matmul(
                    out=psz[:m, :],
                    lhsT=ones_sb[:, :m],
                    rhs=bout_sb[:],
                    start=False,
                    stop=True,
                )
                # ssp(z) = relu(z) + ln(0.5*exp(-|z|) + 0.5): stable for all z
                azs = misc.tile([128, F], FP32, tag="azs")
                nc.scalar.activation(
                    azs[:m, :], psz[:m, :], mybir.ActivationFunctionType.Abs
                )
                ezs = misc.tile([128, F], FP32, tag="ezs")
                nc.scalar.activation(
                    ezs[:m, :],
                    azs[:m, :],
                    mybir.ActivationFunctionType.Exp,
                    scale=-1.0,
                )
                lzs = misc.tile([128, F], FP32, tag="lzs")
                nc.scalar.activation(
                    lzs[:m, :],
                    ezs[:m, :],
                    mybir.ActivationFunctionType.Ln,
                    bias=half_sb[:m, :],
                    scale=0.5,
                )
                rzs = misc.tile([128, F], FP32, tag="rzs")
                nc.vector.tensor_scalar_max(rzs[:m, :], psz[:m, :], 0.0)
                zsb = misc.tile([128, F], FP32)
                nc.vector.tensor_add(zsb[:m, :], lzs[:m, :], rzs[:m, :])
                nc.sync.dma_start(out=out[b, h0 : h0 + m, :], in_=zsb[:m, :])

        def ssp_block(ft_ap, sblk_n):
            # filter net part 1 + ssp over an sblk-group supertile
            ps1 = ps1p.tile([F, sblk * 512], FP32)
            for gl in range(sblk_n):
                nc.tensor.matmul(
                    out=ps1[:, gl * 512 : gl * 512 + GROUP],
                    lhsT=wf1_sb[:],
                    rhs=ft_ap[:, gl * GROUP : (gl + 1) * GROUP],
                    start=True,
                    stop=True,
                )
            act1 = actp.tile([F, sblk * GROUP], BF16D)
            if ssp_mode == "expln":
                # ssp(v) = ln(0.5*exp(v) + 0.5), exact incl. the -ln2
                e_sb = esp.tile([F, sblk * GROUP], FP32)
                nc.scalar.activation(
                    e_sb[:].rearrange("p (c w) -> p c w", w=GROUP),
                    ps1[:].rearrange("p (c w) -> p c w", w=512)[:, :, :GROUP],
                    mybir.ActivationFunctionType.Exp,
                    bias=bf1_sb[:],
                )
                nc.scalar.activation(
                    act1[:],
                    e_sb[:],
                    mybir.ActivationFunctionType.Ln,
                    bias=half_sb[:],
                    scale=0.5,
                )
            else:
                nc.vector.memset(act1[:], 0.25)
            return act1

        def emit_onehot():
            # per-batch 128-row y chunk tables in SBUF (y = x @ Win, bf16)
            ychunks = []
            for bb in range(n_batch):
                for c in range(cpb):
                    r0 = bb * n_atoms + c * 128
                    m = min(128, n_atoms - c * 128)
                    psy = ps2p.tile([128, F], FP32, tag="ps2", name="psy")
                    nc.tensor.matmul(
                        out=psy[:m, :],
                        lhsT=xT_sb[:, r0 : r0 + m],
                        rhs=win_sb[:],
                        start=True,
                        stop=True,
                    )
                    yh = yhp.tile([128, F], BF16D, name=f"yh{r0}")
                    nc.scalar.activation(
                        yh[:m, :], psy[:m, :], mybir.ActivationFunctionType.Copy
                    )
                    ychunks.append((yh, m))

            # Software pipeline: stage s = one supertile (sblk groups,
            # sblk*GROUP pairs). Emission per stage: compares + gather
            # matmuls + mm1 + Exp/Ln for stage s, then mm2 + (+bf2 add) +
            # multiply + per-atom windowed reduce for stage s-1. Keeps each
            # engine's in-order queue free of same-stage cross-engine round
            # trips (naive per-group order serialized everything: 1.04 ms).
            # All DVE/ACT ops are supertile-wide to amortize dispatch.
            SW = sblk * GROUP
            pend = []           # (act1, ynps, ycols, g0)

            def drain():
                while pend:
                    act1, ynps, pycols, g0 = pend.pop(0)
                    ps2 = ps2p.tile([F, sblk * 512], FP32, tag="ps2", name="ps2")
                    for gl in range(sblk):
                        nc.tensor.matmul(
                            out=ps2[:, gl * 512 : gl * 512 + GROUP],
                            lhsT=wf2_sb[:],
                            rhs=act1[:, gl * GROUP : (gl + 1) * GROUP],
                            start=True,
                            stop=True,
                        )
                    # W = ps2 + bf2 -> SBUF bf16 (HW: an op may read only one
                    # PSUM operand, and ynps must stay in PSUM)
                    wsb = sttp.tile([F, SW], BF16D, tag="wsb", name="wsb")
                    nc.vector.tensor_scalar(
                        out=wsb[:].rearrange("p (c w) -> p c w", w=GROUP),
                        in0=ps2[:].rearrange("p (c w) -> p c w", w=512)[:, :, :GROUP],
                        scalar1=bf2p_sb[:],
                        scalar2=None,
                        op0=mybir.AluOpType.add,
                    )
                    prod = sttp.tile([F, SW], BF16D, tag="prod", name="prod")
                    nc.vector.tensor_tensor(
                        out=prod[:].rearrange("p (c w) -> p c w", w=GROUP),
                        in0=wsb[:].rearrange("p (c w) -> p c w", w=GROUP),
                        in1=ynps[:].rearrange("p (c w) -> p c w", w=512)[:, :, :GROUP],
                        op=mybir.AluOpType.mult,
                    )
                    a0 = g0 * ATOMS_PER_GROUP
                    na = sblk * ATOMS_PER_GROUP
                    nc.vector.tensor_reduce(
                        out=pycols[:, a0 : a0 + na],
                        in_=prod[:].rearrange("p (a w) -> p a w", w=NBH),
                        axis=mybir.AxisListType.X,
                        op=mybir.AluOpType.add,
                    )

            for b in range(n_batch):
                ycols = ycolp.tile([F, n_atoms], FP32)
                for gb in range(ng_b // gblk):
                    p0 = (b * ng_b + gb * gblk) * GROUP
                    ibc = ibcp.tile([128, gblk * GROUP], BF16D)
                    nc.sync.dma_start(out=ibc[:], in_=idxbc[:, p0 : p0 + gblk * GROUP])
                    ftg = ftp.tile([G, gblk * GROUP], BF16D)
                    nc.sync.dma_start(out=ftg[:], in_=fT[:, p0 : p0 + gblk * GROUP])
                    for sb in range(gblk // sblk):
                        so = sb * sblk * GROUP
                        g0 = gb * gblk + sb * sblk
                        # stage s: supertile-wide one-hot compares, then
                        # gather matmuls batched per y-chunk (one weight
                        # load per chunk per stage)
                        ohs = []
                        for c in range(cpb):
                            oh = ohp.tile([128, SW], BF16D, tag=f"c{c}")
                            nc.vector.tensor_scalar(
                                out=oh[:],
                                in0=ibc[:, so : so + SW],
                                scalar1=iota_sb[:, c : c + 1],
                                scalar2=None,
                                op0=mybir.AluOpType.is_equal,
                            )
                            ohs.append(oh)
                        ynps = ynpp.tile([F, sblk * 512], FP32)
                        for c in range(cpb):
                            yh, m = ychunks[b * cpb + c]
                            for gl in range(sblk):
                                nc.tensor.matmul(
                                    out=ynps[:, gl * 512 : gl * 512 + GROUP],
                                    lhsT=yh[:m, :],
                                    rhs=ohs[c][:m, gl * GROUP : (gl + 1) * GROUP],
                                    start=(c == 0),
                                    stop=(c == cpb - 1),
                                )
                        # stage s: filter net mm1 + Exp + Ln
                        act1 = ssp_block(ftg[:, so : so + SW], sblk)
                        # stage s-1: mm2 + bias add + mult + reduce
                        cur = (act1, ynps, ycols, g0)
                        drain()
                        pend.append(cur)
                drain()
                f2out(b, ycols)

        def emit_dma():
            # ---- y table (y = x @ Win, bf16, + zero row) ----
            table = dram.tile([n_rows + 1, F], BF16D)
            nc.sync.dma_start(out=table[zrow_id : zrow_id + 1, :], in_=zrow_sb[:])
            for r0 in range(0, n_rows, 128):
                m = min(128, n_rows - r0)
                psy = psmisc.tile([128, F], FP32, tag="pmisc")
                nc.tensor.matmul(
                    out=psy[:m, :],
                    lhsT=xT_sb[:, r0 : r0 + m],
                    rhs=win_sb[:],
                    start=True,
                    stop=True,
                )
                ysb = misc.tile([128, F], BF16D)
                nc.scalar.activation(
                    ysb[:m, :], psy[:m, :], mybir.ActivationFunctionType.Copy
                )
                nc.sync.dma_start(out=table[r0 : r0 + m, :], in_=ysb[:m, :])

            for b in range(n_batch):
                ycols = ycolp.tile([F, n_atoms], FP32)
                for gb in range(ng_b // gblk):
                    ynb = ynbp.tile([F, gblk * IDXW], BF16D)
                    ic0 = (b * ng_b + gb * gblk) * IDXC
                    nc.gpsimd.dma_gather(
                        out_ap=ynb[:].rearrange("p (a n) -> p a n", a=1),
                        in_ap=table[:],
                        idxs_ap=idx_sb[:, ic0 : ic0 + gblk * IDXC],
                        num_idxs=gblk * IDXW,
                        num_idxs_reg=gblk * IDXW,
                        elem_size=F,
                        transpose=True,
                        single_packet=single_packet,
                        queue_num=(b * (ng_b // gblk) + gb) % n_queues,
                    )
                    for sb in range(gblk // sblk):
                        g0 = gb * gblk + sb * sblk
                        p0 = (b * ng_b + g0) * GROUP
                        ft_sb = ftp.tile([G, sblk * GROUP], BF16D)
                        nc.sync.dma_start(
                            out=ft_sb[:], in_=fT[:, p0 : p0 + sblk * GROUP]
                        )
                        act1 = ssp_block(ft_sb[:], sblk)
                        for gl in range(sblk):
                            g = g0 + gl
                            ps2 = ps2p.tile([F, GROUP], FP32)
                            nc.tensor.matmul(
                                out=ps2[:],
                                lhsT=wf2_sb[:],
                                rhs=act1[:, gl * GROUP : (gl + 1) * GROUP],
                                start=True,
                                stop=True,
                            )
                            stt = sttp.tile([F, GROUP], BF16D)
                            for a in range(ATOMS_PER_GROUP):
                                sofs = a * NBH
                                atom = g * ATOMS_PER_GROUP + a
                                yofs = (g - gb * gblk) * IDXW + sofs
                                nc.vector.scalar_tensor_tensor(
                                    out=stt[:, sofs : sofs + NBH],
                                    in0=ps2[:, sofs : sofs + NBH],
                                    scalar=bf2p_sb[:],
                                    in1=ynb[:, yofs : yofs + NBH],
                                    op0=mybir.AluOpType.add,
                                    op1=mybir.AluOpType.mult,
                                    accum_out=ycols[:, atom : atom + 1],
                                )
                f2out(b, ycols)

        emit = emit_onehot if gather_mode == "onehot" else emit_dma
        if repeat == 1:
            emit()
        else:
            with tc.For_i(0, repeat, 1):
                emit()

    nc.compile()
    return nc


def _prep_core(c, x, neighbors, pairwise_mask, f_ij, weights, n_batch=BPC):
    """Host-side marshalling for one core: layouts, casts, index fusion."""
    b0 = c * n_batch
    sl = slice(b0, b0 + n_batch)
    n_atoms = x.shape[1]
    pairs_b = n_atoms * NBH
    ng_b = pairs_b // GROUP
    n_rows = n_batch * n_atoms

    fT = np.ascontiguousarray(
        f_ij[sl].reshape(n_batch * pairs_b, G).T.astype(BF16)
    )
    xT = np.ascontiguousarray(
        x[sl].reshape(n_rows, F).T.astype(BF16)
    )

    nbr = neighbors[sl].astype(np.int64)
    msk = pairwise_mask[sl]

    # onehot path: neighbor id (within batch) bcast over 128 partitions,
    # masked pairs -> 300 (matches no iota row)
    vals = np.where(msk > 0, nbr, 300).reshape(n_batch * pairs_b)
    idxbc = np.ascontiguousarray(
        np.broadcast_to(vals[None, :].astype(BF16), (128, n_batch * pairs_b))
    )
    cpb = (n_atoms + 127) // 128
    iota = np.empty((128, cpb), np.float32)
    for cc in range(cpb):
        iota[:, cc] = np.arange(128) + cc * 128
    iota = np.ascontiguousarray(iota)

    # dma path: batch-offset indices, masked -> zero row
    boff = (np.arange(n_batch) * n_atoms).reshape(n_batch, 1, 1)
    idxm = np.where(msk > 0, nbr + boff, n_rows)
    idxg = idxm.reshape(n_batch * ng_b, GROUP)
    idxp = np.full((n_batch * ng_b, IDXW), n_rows, dtype=np.int64)
    idxp[:, :GROUP] = idxg
    idx16 = (
        idxp.reshape(n_batch * ng_b, IDXC, 16)
        .transpose(2, 0, 1)
        .reshape(16, n_batch * ng_b * IDXC)
        .astype(np.int16)
    )
    idx16 = np.ascontiguousarray(np.tile(idx16, (8, 1)))

    return dict(
        fT=fT,
        xT=xT,
        idx=idx16,
        idxbc=idxbc,
        iota=iota,
        **dict(weights),
    )


def make_in_maps(inputs):
    x = np.asarray(inputs["x"], np.float32)
    f_ij = np.asarray(inputs["f_ij"], np.float32)
    pairwise_mask = np.asarray(inputs["pairwise_mask"], np.float32)
    neighbors = np.asarray(inputs["neighbors"])
    Wf2 = np.asarray(inputs["Wf2"], np.float32)
    weights = dict(
        wf1=np.ascontiguousarray(np.asarray(inputs["Wf1"], np.float32).astype(BF16)),
        wf2=np.ascontiguousarray(Wf2.astype(BF16)),
        win=np.ascontiguousarray(np.asarray(inputs["Win"], np.float32).astype(BF16)),
        wout=np.ascontiguousarray(np.asarray(inputs["Wout"], np.float32).astype(BF16)),
        bf1=np.ascontiguousarray(np.asarray(inputs["bf1"], np.float32).reshape(F, 1)),
        bf2p=np.ascontiguousarray(np.asarray(inputs["bf2"], np.float32).reshape(F, 1)),
        bout=np.ascontiguousarray(
            np.asarray(inputs["bout"], np.float32).astype(BF16).reshape(1, F)
        ),
    )
    return [
        _prep_core(c, x, neighbors, pairwise_mask, f_ij, weights)
        for c in range(NCORES)
    ]


def assemble(results):
    outs = [results[c]["out"] for c in range(NCORES)]
    return np.concatenate(outs, axis=0).reshape(B, N, F).astype(np.float32)


def kernel(
    x,
    r_ij,
    neighbors,
    pairwise_mask,
    f_ij,
    Wf1,
    bf1,
    Wf2,
    bf2,
    Win,
    Wout,
    bout,
):
    inputs = dict(
        x=x, neighbors=neighbors, pairwise_mask=pairwise_mask, f_ij=f_ij,
        Wf1=Wf1, bf1=bf1, Wf2=Wf2, bf2=bf2, Win=Win, Wout=Wout, bout=bout,
    )
    nc = build_nc()
    in_maps = make_in_maps(inputs)
    res = run_bass_kernel_spmd(
        nc,
        in_maps,
        core_ids=list(range(NCORES)),
        trace=bool(int(os.environ.get("CFCONV_TRACE", "0"))),
    )
    kernel.last_results = res
    return assemble(res.results)


# revision 35
# speedup vs baseline: 1.8569x; 1.8569x over previous
"""SchNet CFConv kernel for 8 TRN2 NeuronCores (Bass/Tile).

Math (per batch b, atom n, neighbor slot k):
    W   = ssp(f_ij @ Wf1 + bf1) @ Wf2 + bf2          ssp(v) = softplus(v) - ln2
    y   = x @ Win
    out = ssp( (sum_k mask * W * y[nbr]) @ Wout + bout )

Device strategy (data-parallel over batch, 2 molecules per core):
  * Host pre-transposes f_ij to fT [G, pairs] bf16 so the filter matmuls run
    with G on partitions (no on-device transposes).
  * ssp(v) = ln(0.5*exp(v)+0.5) exactly (no Softplus table in this
    toolchain; Exp/Ln/Abs/Copy share the natural_log_exp table set). The
    final layer uses the stable relu(z) + ln(0.5*exp(-|z|)+0.5) form.
  * Neighbor gather = one-hot matmul on TensorE (gather_mode="onehot",
    default): host uploads the neighbor index row replicated across 128
    partitions as bf16 (masked pairs -> 300, matches nothing). DVE builds
    one-hot tiles with is_equal against per-partition iota constants
    (rows 0-127 / 128-255), and two accumulated K=128 matmuls against the
    per-batch y half-tables select y[nbr] exactly into PSUM. This avoids
    gpsimd dma_gather entirely, whose single-Q7-pair descriptor generation
    (~14 ns/idx) dominated the old kernel (~1.9 ms of the 2.0 ms total).
  * gather_mode="dma" keeps the old dma_gather path for comparison.
"""

import math
import os
from contextlib import ExitStack

import ml_dtypes
import numpy as np

import concourse.bass as bass
import concourse.mybir as mybir
import concourse.tile as tile
from concourse import bacc, library_config
from concourse.bass_utils import run_bass_kernel_spmd

BF16 = ml_dtypes.bfloat16
LOG2 = float(np.log(2.0))

B, N, NBH, G, F = 16, 256, 255, 50, 128
NCORES = 8
BPC = B // NCORES          # batches (molecules) per core
PAIRS_B = N * NBH          # 65280 pairs per batch
ATOMS_PER_GROUP = 2
GROUP = ATOMS_PER_GROUP * NBH   # 510 pairs per group
NG_B = PAIRS_B // GROUP         # 128 groups per batch
IDXW = 512                      # gather idxs per group (510 real + 2 pad)
IDXC = IDXW // 16               # idx columns per group in the [16, *] layout

FP32 = mybir.dt.float32
BF16D = mybir.dt.bfloat16
I16 = mybir.dt.int16


def build_nc(n_batch=BPC, n_atoms=N, repeat=1, gather_mode="onehot",
             single_packet=False, ssp_mode="expln", n_queues=1, gblk=16,
             skip=(), evac_w="alt", evac_y="none", splits=None):
    """Build the per-core Bass program. Parametric so CoreSim can run tiny.

    ssp_mode: "expln" (production, 2 ACT passes), "zero" (no ACT in main
    loop, timing experiment only).
    """
    assert n_atoms % ATOMS_PER_GROUP == 0
    pairs_b = n_atoms * NBH
    ng_b = pairs_b // GROUP           # groups per batch
    n_rows = n_batch * n_atoms        # y-table rows (+1 zero row for dma mode)
    zrow_id = n_rows
    cpb = (n_atoms + 127) // 128      # 128-row y chunks per batch (onehot)

    nc = bacc.Bacc(None, target_bir_lowering=False, num_swdge_queues=n_queues)

    fT = nc.declare_dram_parameter("fT", [G, n_batch * pairs_b], BF16D, False)
    xT = nc.declare_dram_parameter("xT", [F, n_rows], BF16D, False)
    wf1 = nc.declare_dram_parameter("wf1", [G, F], BF16D, False)
    wf2 = nc.declare_dram_parameter("wf2", [F, F], BF16D, False)
    win = nc.declare_dram_parameter("win", [F, F], BF16D, False)
    wout = nc.declare_dram_parameter("wout", [F, F], BF16D, False)
    bf1 = nc.declare_dram_parameter("bf1", [F, 1], FP32, False)
    bf2p = nc.declare_dram_parameter("bf2p", [F, 1], FP32, False)
    bout = nc.declare_dram_parameter("bout", [1, F], BF16D, False)
    if gather_mode == "onehot":
        idxbc = nc.declare_dram_parameter(
            "idxbc", [128, n_batch * pairs_b], BF16D, False
        )
        iota = nc.declare_dram_parameter("iota", [128, cpb], FP32, False)
    else:
        idx = nc.declare_dram_parameter(
            "idx", [128, n_batch * ng_b * IDXC], I16, False
        )
    out = nc.declare_dram_parameter("out", [n_batch, n_atoms, F], FP32, isOutput=True)

    with tile.TileContext(nc) as tc, ExitStack() as ctx:
        # Pre-load ACT table set 6 (natural_log_exp_and_others): it contains
        # every function this kernel uses (Exp, Ln, Copy, Identity, Abs,
        # Relu). Without this, insert_act_table_loads picks the FIRST set
        # containing each func (Exp->0, Ln->5) and the alternating Exp/Ln
        # stream reloads tables every supertile: 260 loads = 348 us of ACT.
        nc.scalar.add_instruction(
            mybir.InstLoadActFuncSet(
                name=nc.get_next_instruction_name(),
                ins=[],
                outs=[],
                act_func_set_id=6,
            )
        )
        consts = ctx.enter_context(tc.tile_pool(name="consts", bufs=1))
        misc = ctx.enter_context(tc.tile_pool(name="misc", bufs=4))
        ftp = ctx.enter_context(tc.tile_pool(name="ftp", bufs=2))
        esp = ctx.enter_context(tc.tile_pool(name="esp", bufs=2))
        actp = ctx.enter_context(tc.tile_pool(name="actp", bufs=4))
        sttp = ctx.enter_context(tc.tile_pool(name="sttp", bufs=2))
        ps1p = ctx.enter_context(tc.tile_pool(name="ps1p", bufs=1, space="PSUM"))
        ps2p = ctx.enter_context(tc.tile_pool(name="ps2p", bufs=1, space="PSUM"))
        ycolp = ctx.enter_context(tc.tile_pool(name="ycolp", bufs=2))
        yfinp = ctx.enter_context(tc.tile_pool(name="yfinp", bufs=2))
        if gather_mode == "onehot":
            ibcp = ctx.enter_context(tc.tile_pool(name="ibcp", bufs=2))
            ohp = ctx.enter_context(tc.tile_pool(name="ohp", bufs=2 * cpb))
            ynpp = ctx.enter_context(tc.tile_pool(name="ynpp", bufs=2, space="PSUM"))
            yhp = ctx.enter_context(tc.tile_pool(name="yhp", bufs=n_batch * cpb))
        else:
            dram = ctx.enter_context(tc.tile_pool(name="dram", bufs=1, space="DRAM"))
            psmisc = ctx.enter_context(tc.tile_pool(name="psmisc", bufs=1, space="PSUM"))
            ynbp = ctx.enter_context(tc.tile_pool(name="ynbp", bufs=3))
            nc.gpsimd.load_library(library_config.mlp)

        # ---- constants into SBUF ----
        wf1_sb = consts.tile([G, F], BF16D)
        nc.sync.dma_start(out=wf1_sb[:], in_=wf1[:])
        wf2_sb = consts.tile([F, F], BF16D)
        nc.sync.dma_start(out=wf2_sb[:], in_=wf2[:])
        win_sb = consts.tile([F, F], BF16D)
        nc.sync.dma_start(out=win_sb[:], in_=win[:])
        wout_sb = consts.tile([F, F], BF16D)
        nc.sync.dma_start(out=wout_sb[:], in_=wout[:])
        bf1_sb = consts.tile([F, 1], FP32)
        nc.sync.dma_start(out=bf1_sb[:], in_=bf1[:])
        bf2p_sb = consts.tile([F, 1], FP32)
        nc.sync.dma_start(out=bf2p_sb[:], in_=bf2p[:])
        bout_sb = consts.tile([1, F], BF16D)
        nc.sync.dma_start(out=bout_sb[:], in_=bout[:])
        xT_sb = consts.tile([F, n_rows], BF16D)
        nc.sync.dma_start(out=xT_sb[:], in_=xT[:])
        ones_sb = consts.tile([1, F], BF16D)
        nc.vector.memset(ones_sb[:], 1.0)
        half_sb = consts.tile([F, 1], FP32)
        nc.vector.memset(half_sb[:], 0.5)
        if gather_mode == "onehot":
            iota_sb = consts.tile([128, cpb], FP32)
            nc.sync.dma_start(out=iota_sb[:], in_=iota[:])
        else:
            zrow_sb = consts.tile([1, F], BF16D)
            nc.vector.memset(zrow_sb[:], 0.0)
            idx_sb = consts.tile([128, n_batch * ng_b * IDXC], I16)
            nc.sync.dma_start(out=idx_sb[:], in_=idx[:])

        sblk = min(2, ng_b)   # groups per ssp supertile
        gblk = min(gblk, ng_b)  # groups per stream block
        assert ng_b % gblk == 0 and gblk % sblk == 0

        psz_pool = ps2p if gather_mode == "onehot" else psmisc
        psz_tag = "ps2" if gather_mode == "onehot" else "pmisc"

        def f2out(b, ycols):
            # out[b] = ssp(ycols.T @ Wout + bout)
            yfin = yfinp.tile([F, n_atoms], BF16D)
            nc.vector.tensor_copy(out=yfin[:], in_=ycols[:])
            for h0 in range(0, n_atoms, 128):
                m = min(128, n_atoms - h0)
                psz = psz_pool.tile([128, F], FP32, tag=psz_tag, name="psz")
                nc.tensor.matmul(
                    out=psz[:m, :],
                    lhsT=yfin[:, h0 : h0 + m],
                    rhs=wout_sb[:],
                    start=True,
                    stop=False,
                )
                nc.tensor.matmul(
                    out=psz[:m, :],
                    lhsT=ones_sb[:, :m],
                    rhs=bout_sb[:],
                    start=False,
                    stop=True,
                )
                # ssp(z) = relu(z) + ln(0.5*exp(-|z|) + 0.5): stable for all z
                azs = misc.tile([128, F], FP32, tag="azs")
                nc.scalar.activation(
                    azs[:m, :], psz[:m, :], mybir.ActivationFunctionType.Abs
                )
                ezs = misc.tile([128, F], FP32, tag="ezs")
                nc.scalar.activation(
                    ezs[:m, :],
                    azs[:m, :],
                    mybir.ActivationFunctionType.Exp,
                    scale=-1.0,
                )
                lzs = misc.tile([128, F], FP32, tag="lzs")
                nc.scalar.activation(
                    lzs[:m, :],
                    ezs[:m, :],
                    mybir.ActivationFunctionType.Ln,
                    bias=half_sb[:m, :],
                    scale=0.5,
                )
                rzs = misc.tile([128, F], FP32, tag="rzs")
                nc.vector.tensor_scalar_max(rzs[:m, :], psz[:m, :], 0.0)
                zsb = misc.tile([128, F], FP32)
                nc.vector.tensor_add(zsb[:m, :], lzs[:m, :], rzs[:m, :])
                nc.sync.dma_start(out=out[b, h0 : h0 + m, :], in_=zsb[:m, :])

        def ssp_block(ft_ap, sblk_n):
            # filter net part 1 + ssp over an sblk-group supertile
            ps1 = ps1p.tile([F, sblk * 512], FP32)
            for gl in range(sblk_n):
                nc.tensor.matmul(
                    out=ps1[:, gl * 512 : gl * 512 + GROUP],
                    lhsT=wf1_sb[:],
                    rhs=ft_ap[:, gl * GROUP : (gl + 1) * GROUP],
                    start=True,
                    stop=True,
                )
            act1 = actp.tile([F, sblk * GROUP], BF16D)
            if ssp_mode == "expln":
                # ssp(v) = ln(0.5*exp(v) + 0.5), exact incl. the -ln2
                e_sb = esp.tile([F, sblk * GROUP], FP32)
                nc.scalar.activation(
                    e_sb[:].rearrange("p (c w) -> p c w", w=GROUP),
                    ps1[:].rearrange("p (c w) -> p c w", w=512)[:, :, :GROUP],
                    mybir.ActivationFunctionType.Exp,
                    bias=bf1_sb[:],
                )
                nc.scalar.activation(
                    act1[:],
                    e_sb[:],
                    mybir.ActivationFunctionType.Ln,
                    bias=half_sb[:],
                    scale=0.5,
                )
            else:
                nc.vector.memset(act1[:], 0.25)
            return act1

        def emit_onehot():
            # per-batch 128-row y chunk tables in SBUF (y = x @ Win, bf16)
            ychunks = []
            for bb in range(n_batch):
                for c in range(cpb):
                    r0 = bb * n_atoms + c * 128
                    m = min(128, n_atoms - c * 128)
                    psy = ps2p.tile([128, F], FP32, tag="ps2", name="psy")
                    nc.tensor.matmul(
                        out=psy[:m, :],
                        lhsT=xT_sb[:, r0 : r0 + m],
                        rhs=win_sb[:],
                        start=True,
                        stop=True,
                    )
                    yh = yhp.tile([128, F], BF16D, name=f"yh{r0}")
                    nc.scalar.activation(
                        yh[:m, :], psy[:m, :], mybir.ActivationFunctionType.Copy
                    )
                    ychunks.append((yh, m))

            # Software pipeline: stage s = one supertile (sblk groups,
            # sblk*GROUP pairs). Emission per stage: compares + gather
            # matmuls + mm1 + Exp/Ln for stage s, then mm2 + (+bf2 add) +
            # multiply + per-atom windowed reduce for stage s-1. Keeps each
            # engine's in-order queue free of same-stage cross-engine round
            # trips (naive per-group order serialized everything: 1.04 ms).
            # All DVE/ACT ops are supertile-wide to amortize dispatch.
            SW = sblk * GROUP
            pend = []           # (act1, ynps, ycols, g0)

            def drain():
                if "drain" in skip:
                    for item in pend:
                        if item[5]:
                            f2out(item[4], item[2])
                    pend.clear()
                    return
                while pend:
                    act1, ynps, pycols, g0, bb_, last_ = pend.pop(0)
                    ps2 = ps2p.tile([F, sblk * 512], FP32, tag="ps2", name="ps2")
                    for gl in range(sblk):
                        nc.tensor.matmul(
                            out=ps2[:, gl * 512 : gl * 512 + GROUP],
                            lhsT=wf2_sb[:],
                            rhs=act1[:, gl * GROUP : (gl + 1) * GROUP],
                            start=True,
                            stop=True,
                        )
                    # Evacuate both PSUM tensors to SBUF bf16 once per stage
                    # (HW: a DVE op may read only one PSUM operand, and the
    	            # per-instruction PSUM access penalty is large). The
                    # per-atom STTs then run all-SBUF bf16, which qualifies
                    # for the DVE 4x perf mode. Each evacuation alternates
                    # between ACT (Identity w/ AP bias) and DVE per stage to
                    # balance engine load (counter-phased).
                    phase = (g0 // sblk) % 2
                    wsb = sttp.tile([F, SW], BF16D, tag="wsb", name="wsb")
                    if (phase == 0) if evac_w == "alt" else (evac_w == "act"):
                        nc.scalar.activation(
                            wsb[:].rearrange("p (c w) -> p c w", w=GROUP),
                            ps2[:].rearrange("p (c w) -> p c w", w=512)[:, :, :GROUP],
                            mybir.ActivationFunctionType.Identity,
                            bias=bf2p_sb[:],
                        )
                    else:
                        nc.vector.tensor_scalar(
                            out=wsb[:].rearrange("p (c w) -> p c w", w=GROUP),
                            in0=ps2[:].rearrange("p (c w) -> p c w", w=512)[:, :, :GROUP],
                            scalar1=bf2p_sb[:],
                            scalar2=None,
                            op0=mybir.AluOpType.add,
                        )
                    if evac_y == "none":
                        ynr = ynps
                        yn_of = lambda gl, a: gl * 512 + a * NBH  # noqa: E731
                    else:
                        ynsb = sttp.tile([F, SW], BF16D, tag="ynsb", name="ynsb")
                        if (phase == 1) if evac_y == "alt" else (evac_y == "act"):
                            nc.scalar.activation(
                                ynsb[:].rearrange("p (c w) -> p c w", w=GROUP),
                                ynps[:].rearrange("p (c w) -> p c w", w=512)[:, :, :GROUP],
                                mybir.ActivationFunctionType.Copy,
                            )
                        else:
                            nc.vector.tensor_copy(
                                out=ynsb[:].rearrange("p (c w) -> p c w", w=GROUP),
                                in_=ynps[:].rearrange("p (c w) -> p c w", w=512)[:, :, :GROUP],
                            )
                        ynr = ynsb
                        yn_of = lambda gl, a: gl * GROUP + a * NBH  # noqa: E731
                    # per-atom multiply + reduce fused in one STT (accum_out)
                    stt = sttp.tile([F, SW], BF16D, tag="stt", name="stt")
                    for gl in range(sblk):
                        for a in range(ATOMS_PER_GROUP):
                            sofs = gl * GROUP + a * NBH
                            atom = (g0 + gl) * ATOMS_PER_GROUP + a
                            nc.vector.scalar_tensor_tensor(
                                out=stt[:, sofs : sofs + NBH],
                                in0=wsb[:, sofs : sofs + NBH],
                                scalar=0.0,
                                in1=ynr[:, yn_of(gl, a) : yn_of(gl, a) + NBH],
                                op0=mybir.AluOpType.add,
                                op1=mybir.AluOpType.mult,
                                accum_out=pycols[:, atom : atom + 1],
                            )
                    if last_:
                        f2out(bb_, pycols)

            # Flat stage list across batches; compares are emitted one full
            # stage AHEAD of their consumers so DVE's in-order queue never
            # parks the next stage's compare behind this stage's drain ops
            # (which wait on TensorE's mm2) — that coupling was the serial
            # critical path.
            nstg = ng_b // sblk
            stage_list = [
                (b, gb, sb)
                for b in range(n_batch)
                for gb in range(ng_b // gblk)
                for sb in range(gblk // sblk)
            ]
            blocks = {}
            ycols_of = {}

            def ensure_block(b, gb):
                if (b, gb) not in blocks:
                    p0 = (b * ng_b + gb * gblk) * GROUP
                    ibc = ibcp.tile([128, gblk * GROUP], BF16D, name="ibc")
                    nc.sync.dma_start(
                        out=ibc[:], in_=idxbc[:, p0 : p0 + gblk * GROUP]
                    )
                    ftg = ftp.tile([G, gblk * GROUP], BF16D, name="ftg")
                    nc.sync.dma_start(
                        out=ftg[:], in_=fT[:, p0 : p0 + gblk * GROUP]
                    )
                    blocks[(b, gb)] = (ibc, ftg)
                return blocks[(b, gb)]

            def emit_cmps(b, gb, sb):
                ibc, _ = ensure_block(b, gb)
                so = sb * sblk * GROUP
                ncmp = 1 if splits is not None else cpb
                ohs = []
                for c in range(ncmp):
                    oh = ohp.tile([128, SW], BF16D, tag=f"c{c}", name="oh")
                    nc.vector.tensor_scalar(
                        out=oh[:],
                        in0=ibc[:, so : so + SW],
                        scalar1=iota_sb[:, c : c + 1],
                        scalar2=None,
                        op0=mybir.AluOpType.is_equal,
                    )
                    ohs.append(oh)
                return ohs

            ohs_next = None
            for i, (b, gb, sb) in enumerate(stage_list):
                _, ftg = ensure_block(b, gb)
                so = sb * sblk * GROUP
                g0 = gb * gblk + sb * sblk
                if sb == 0 and gb == 0:
                    ycols = ycolp.tile([F, n_atoms], FP32, name="ycols")
                    ycols_of[b] = ycols
                    if "drain" in skip:
                        nc.vector.memset(ycols[:], 0.125)
                ycols = ycols_of[b]

                if "gather" in skip:
                    ynps = ynpp.tile([F, sblk * 512], FP32)
                    nc.vector.memset(ynps[:], 0.25)
                else:
                    if ohs_next is None:
                        ohs_next = emit_cmps(b, gb, sb)
                    ohs = ohs_next
                    ynps = ynpp.tile([F, sblk * 512], FP32)
                    if splits is not None:
                        # pairs per atom sorted [nbr<128 | nbr>=128 | masked];
                        # adjusted idx values share one compare, and each
                        # atom's two matmuls select the matching y chunk.
                        # Batched per chunk so lhsT loads once per stage.
                        for c in range(cpb):
                            yh, m = ychunks[b * cpb + c]
                            for gl in range(sblk):
                                for a in range(ATOMS_PER_GROUP):
                                    atom = (g0 + gl) * ATOMS_PER_GROUP + a
                                    w = int(splits[b][atom])
                                    aofs = gl * GROUP + a * NBH
                                    pofs = gl * 512 + a * NBH
                                    lo, hi = (0, w) if c == 0 else (w, NBH)
                                    if hi <= lo:
                                        continue
                                    nc.tensor.matmul(
                                        out=ynps[:, pofs + lo : pofs + hi],
                                        lhsT=yh[:m, :],
                                        rhs=ohs[0][:m, aofs + lo : aofs + hi],
                                        start=True,
                                        stop=True,
                                    )
                    else:
                        for c in range(cpb):
                            yh, m = ychunks[b * cpb + c]
                            for gl in range(sblk):
                                nc.tensor.matmul(
                                    out=ynps[:, gl * 512 : gl * 512 + GROUP],
                                    lhsT=yh[:m, :],
                                    rhs=ohs[c][:m, gl * GROUP : (gl + 1) * GROUP],
                                    start=(c == 0),
                                    stop=(c == cpb - 1),
                                )
                # stage s: filter net mm1 + Exp + Ln
                act1 = ssp_block(ftg[:, so : so + SW], sblk)
                # compares for stage s+1 (ahead of the s-1 drain on DVE)
                if "gather" not in skip and i + 1 < len(stage_list):
                    ohs_next = emit_cmps(*stage_list[i + 1])
                # stage s-1: mm2 + bias add + mult + reduce
                cur = (act1, ynps, ycols, g0, b, sb == gblk // sblk - 1
                       and gb == ng_b // gblk - 1)
                drain()
                pend.append(cur)
            drain()

        def emit_dma():
            # ---- y table (y = x @ Win, bf16, + zero row) ----
            table = dram.tile([n_rows + 1, F], BF16D)
            nc.sync.dma_start(out=table[zrow_id : zrow_id + 1, :], in_=zrow_sb[:])
            for r0 in range(0, n_rows, 128):
                m = min(128, n_rows - r0)
                psy = psmisc.tile([128, F], FP32, tag="pmisc")
                nc.tensor.matmul(
                    out=psy[:m, :],
                    lhsT=xT_sb[:, r0 : r0 + m],
                    rhs=win_sb[:],
                    start=True,
                    stop=True,
                )
                ysb = misc.tile([128, F], BF16D)
                nc.scalar.activation(
                    ysb[:m, :], psy[:m, :], mybir.ActivationFunctionType.Copy
                )
                nc.sync.dma_start(out=table[r0 : r0 + m, :], in_=ysb[:m, :])

            for b in range(n_batch):
                ycols = ycolp.tile([F, n_atoms], FP32)
                for gb in range(ng_b // gblk):
                    ynb = ynbp.tile([F, gblk * IDXW], BF16D)
                    ic0 = (b * ng_b + gb * gblk) * IDXC
                    nc.gpsimd.dma_gather(
                        out_ap=ynb[:].rearrange("p (a n) -> p a n", a=1),
                        in_ap=table[:],
                        idxs_ap=idx_sb[:, ic0 : ic0 + gblk * IDXC],
                        num_idxs=gblk * IDXW,
                        num_idxs_reg=gblk * IDXW,
                        elem_size=F,
                        transpose=True,
                        single_packet=single_packet,
                        queue_num=(b * (ng_b // gblk) + gb) % n_queues,
                    )
                    for sb in range(gblk // sblk):
                        g0 = gb * gblk + sb * sblk
                        p0 = (b * ng_b + g0) * GROUP
                        ft_sb = ftp.tile([G, sblk * GROUP], BF16D)
                        nc.sync.dma_start(
                            out=ft_sb[:], in_=fT[:, p0 : p0 + sblk * GROUP]
                        )
                        act1 = ssp_block(ft_sb[:], sblk)
                        for gl in range(sblk):
                            g = g0 + gl
                            ps2 = ps2p.tile([F, GROUP], FP32)
                            nc.tensor.matmul(
                                out=ps2[:],
                                lhsT=wf2_sb[:],
                                rhs=act1[:, gl * GROUP : (gl + 1) * GROUP],
                                start=True,
                                stop=True,
                            )
                            stt = sttp.tile([F, GROUP], BF16D)
                            for a in range(ATOMS_PER_GROUP):
                                sofs = a * NBH
                                atom = g * ATOMS_PER_GROUP + a
                                yofs = (g - gb * gblk) * IDXW + sofs
                                nc.vector.scalar_tensor_tensor(
                                    out=stt[:, sofs : sofs + NBH],
                                    in0=ps2[:, sofs : sofs + NBH],
                                    scalar=bf2p_sb[:],
                                    in1=ynb[:, yofs : yofs + NBH],
                                    op0=mybir.AluOpType.add,
                                    op1=mybir.AluOpType.mult,
                                    accum_out=ycols[:, atom : atom + 1],
                                )
                f2out(b, ycols)

        emit = emit_onehot if gather_mode == "onehot" else emit_dma
        if repeat == 1:
            emit()
        else:
            with tc.For_i(0, repeat, 1):
                emit()

    nc.compile()
    return nc


def _prep_core(c, x, neighbors, pairwise_mask, f_ij, weights, n_batch=BPC):
    """Host-side marshalling for one core: layouts, casts, index fusion."""
    b0 = c * n_batch
    sl = slice(b0, b0 + n_batch)
    n_atoms = x.shape[1]
    pairs_b = n_atoms * NBH
    ng_b = pairs_b // GROUP
    n_rows = n_batch * n_atoms

    fT = np.ascontiguousarray(
        f_ij[sl].reshape(n_batch * pairs_b, G).T.astype(BF16)
    )
    xT = np.ascontiguousarray(
        x[sl].reshape(n_rows, F).T.astype(BF16)
    )

    nbr = neighbors[sl].astype(np.int64)
    msk = pairwise_mask[sl]

    # onehot path: neighbor id (within batch) bcast over 128 partitions,
    # masked pairs -> 300 (matches no iota row)
    vals = np.where(msk > 0, nbr, 300).reshape(n_batch * pairs_b)
    idxbc = np.ascontiguousarray(
        np.broadcast_to(vals[None, :].astype(BF16), (128, n_batch * pairs_b))
    )
    cpb = (n_atoms + 127) // 128
    iota = np.empty((128, cpb), np.float32)
    for cc in range(cpb):
        iota[:, cc] = np.arange(128) + cc * 128
    iota = np.ascontiguousarray(iota)

    # dma path: batch-offset indices, masked -> zero row
    boff = (np.arange(n_batch) * n_atoms).reshape(n_batch, 1, 1)
    idxm = np.where(msk > 0, nbr + boff, n_rows)
    idxg = idxm.reshape(n_batch * ng_b, GROUP)
    idxp = np.full((n_batch * ng_b, IDXW), n_rows, dtype=np.int64)
    idxp[:, :GROUP] = idxg
    idx16 = (
        idxp.reshape(n_batch * ng_b, IDXC, 16)
        .transpose(2, 0, 1)
        .reshape(16, n_batch * ng_b * IDXC)
        .astype(np.int16)
    )
    idx16 = np.ascontiguousarray(np.tile(idx16, (8, 1)))

    return dict(
        fT=fT,
        xT=xT,
        idx=idx16,
        idxbc=idxbc,
        iota=iota,
        **dict(weights),
    )


def make_in_maps(inputs):
    x = np.asarray(inputs["x"], np.float32)
    f_ij = np.asarray(inputs["f_ij"], np.float32)
    pairwise_mask = np.asarray(inputs["pairwise_mask"], np.float32)
    neighbors = np.asarray(inputs["neighbors"])
    Wf2 = np.asarray(inputs["Wf2"], np.float32)
    weights = dict(
        wf1=np.ascontiguousarray(np.asarray(inputs["Wf1"], np.float32).astype(BF16)),
        wf2=np.ascontiguousarray(Wf2.astype(BF16)),
        win=np.ascontiguousarray(np.asarray(inputs["Win"], np.float32).astype(BF16)),
        wout=np.ascontiguousarray(np.asarray(inputs["Wout"], np.float32).astype(BF16)),
        bf1=np.ascontiguousarray(np.asarray(inputs["bf1"], np.float32).reshape(F, 1)),
        bf2p=np.ascontiguousarray(np.asarray(inputs["bf2"], np.float32).reshape(F, 1)),
        bout=np.ascontiguousarray(
            np.asarray(inputs["bout"], np.float32).astype(BF16).reshape(1, F)
        ),
    )
    return [
        _prep_core(c, x, neighbors, pairwise_mask, f_ij, weights)
        for c in range(NCORES)
    ]


def assemble(results):
    outs = [results[c]["out"] for c in range(NCORES)]
    return np.concatenate(outs, axis=0).reshape(B, N, F).astype(np.float32)


def kernel(
    x,
    r_ij,
    neighbors,
    pairwise_mask,
    f_ij,
    Wf1,
    bf1,
    Wf2,
    bf2,
    Win,
    Wout,
    bout,
):
    inputs = dict(
        x=x, neighbors=neighbors, pairwise_mask=pairwise_mask, f_ij=f_ij,
        Wf1=Wf1, bf1=bf1, Wf2=Wf2, bf2=bf2, Win=Win, Wout=Wout, bout=bout,
    )
    nc = build_nc()
    in_maps = make_in_maps(inputs)
    res = run_bass_kernel_spmd(
        nc,
        in_maps,
        core_ids=list(range(NCORES)),
        trace=bool(int(os.environ.get("CFCONV_TRACE", "0"))),
    )
    kernel.last_results = res
    return assemble(res.results)
